# revision 1
# baseline (speedup 1.0000x reference)
"""Trainium2 Bass kernel for DeepFunnelTransactionMLP.

MLP funnel 15->30->60->90->120->90->60->30->15->10->5->1 (ReLU between,
sigmoid at the end) over a batch of 524288 rows, fp32.

Strategy
--------
Pure data parallel: 8 cores x 65536 rows. On each core, activations are
kept feature-major (features on SBUF partitions, batch streaming on the
free dim), so every layer is one (or two) matmul(s) with the weight as
the stationary operand. Small layers are packed block-diagonally: e.g.
layer1 (15->30) processes 4 independent batch chunks in a single matmul
(4x15 input rows -> 4x30 output rows). Bias+ReLU are fused into single
ScalarE activation / VectorE tensor_scalar instructions reading PSUM.

Host side does the free work: transposing/packing x, building the
block-diagonal weights, and unpermuting the output.
"""

import sys

sys.path.insert(0, "/opt/trn_rl_repo")

import numpy as np

import concourse.bacc as bacc
import concourse.mybir as mybir
from concourse.bass_utils import run_bass_kernel_spmd
from concourse.tile import TileContext

_DIMS = [15, 30, 60, 90, 120, 90, 60, 30, 15, 10, 5, 1]
NCORES = 8
B = 524288
BC = B // NCORES  # 65536 rows per core
S = 4096  # super-tile rows
NST = BC // S  # 16 super-tiles per core
F32 = mybir.dt.float32

# Weight variants: (layer l (1-based), K, M, [(koff, moff), ...]).
# lhsT[koff+k, moff+m] = W_l[m, k] for each block; matmul out = lhsT.T @ rhs.
_VARIANTS = [
    ("w1", 1, 60, 120, [(15 * j, 30 * j) for j in range(4)]),
    ("w2A", 2, 60, 120, [(0, 0), (30, 60)]),
    ("w2B", 2, 120, 120, [(60, 0), (90, 60)]),
    ("w3A", 3, 60, 90, [(0, 0)]),
    ("w3B", 3, 120, 90, [(60, 0)]),
    ("w4", 4, 90, 120, [(0, 0)]),
    ("w5", 5, 120, 90, [(0, 0)]),
    ("w6A", 6, 90, 60, [(0, 0)]),
    ("w6B", 6, 90, 120, [(0, 60)]),
    ("w7A", 7, 120, 60, [(0, 0), (60, 30)]),
    ("w7B", 7, 120, 120, [(0, 60), (60, 90)]),
    ("w8A", 8, 120, 60, [(30 * j, 15 * j) for j in range(4)]),
    ("w8B", 8, 120, 120, [(30 * j, 60 + 15 * j) for j in range(4)]),
    ("w9", 9, 120, 80, [(15 * j, 10 * j) for j in range(8)]),
    ("w10", 10, 80, 40, [(10 * j, 5 * j) for j in range(8)]),
    ("w11", 11, 40, 8, [(5 * j, j) for j in range(8)]),
]
_VIDX = {name: i for i, (name, *_) in enumerate(_VARIANTS)}

# Bias layouts: (layer l, tile count) -> packed [tile*dim] at column l-1.
_BIAS_TILES = [4, 2, 1, 1, 1, 2, 4, 8, 8, 8, 8]


def _pack_weights(Ws):
    w = np.zeros((128, 128 * len(_VARIANTS)), dtype=np.float32)
    for i, (_, l, K, M, blocks) in enumerate(_VARIANTS):
        Wl = Ws[l - 1]  # [fan_out, fan_in]
        fo, fi = Wl.shape
        for koff, moff in blocks:
            w[koff : koff + fi, 128 * i + moff : 128 * i + moff + fo] = Wl.T
    return w


def _pack_biases(bs):
    b = np.zeros((128, 16), dtype=np.float32)
    for l, (bl, nt) in enumerate(zip(bs, _BIAS_TILES)):
        v = np.tile(bl, nt)
        b[: v.shape[0], l] = v
    return b


def _out_map():
    """batch-row (within a super-tile) for output element [group j, col n]."""
    M0 = np.arange(S).reshape(4, S // 4)
    M1 = M0
    M2 = np.empty((2, 2048), dtype=np.int64)
    for t in range(2):
        M2[:, 512 * t : 512 * (t + 1)] = M1[0:2, 512 * t : 512 * (t + 1)]
        M2[:, 1024 + 512 * t : 1024 + 512 * (t + 1)] = M1[2:4, 512 * t : 512 * (t + 1)]
    M3 = np.empty((1, 4096), dtype=np.int64)
    for u in range(4):
        M3[0, 512 * u : 512 * (u + 1)] = M2[0, 512 * u : 512 * (u + 1)]
        M3[0, 2048 + 512 * u : 2048 + 512 * (u + 1)] = M2[1, 512 * u : 512 * (u + 1)]
    M5 = M3
    M6 = np.empty((2, 2048), dtype=np.int64)
    for w in range(4):
        M6[0, 512 * w : 512 * (w + 1)] = M5[0, 1024 * w : 1024 * w + 512]
        M6[1, 512 * w : 512 * (w + 1)] = M5[0, 1024 * w + 512 : 1024 * w + 1024]
    M7 = np.empty((4, 1024), dtype=np.int64)
    for w in range(2):
        M7[0:2, 512 * w : 512 * (w + 1)] = M6[0:2, 1024 * w : 1024 * w + 512]
        M7[2:4, 512 * w : 512 * (w + 1)] = M6[0:2, 1024 * w + 512 : 1024 * w + 1024]
    M8 = np.empty((8, 512), dtype=np.int64)
    M8[0:4, :] = M7[0:4, 0:512]
    M8[4:8, :] = M7[0:4, 512:1024]
    return M8


_NC_CACHE = None


def _build_nc():
    global _NC_CACHE
    if _NC_CACHE is not None:
        return _NC_CACHE

    nc = bacc.Bacc("TRN2", target_bir_lowering=False, debug=False, num_devices=NCORES)
    xt = nc.dram_tensor("xt", [60, BC // 4], F32, kind="ExternalInput")
    wd = nc.dram_tensor("w", [128, 128 * len(_VARIANTS)], F32, kind="ExternalInput")
    bd = nc.dram_tensor("b", [128, 16], F32, kind="ExternalInput")
    y = nc.dram_tensor("y", [8, BC // 8], F32, kind="ExternalOutput")

    with TileContext(nc) as tc:
        with (
            tc.tile_pool(name="const", bufs=1) as cpool,
            tc.tile_pool(name="act", bufs=1) as apool,
            tc.tile_pool(name="io", bufs=2) as iopool,
            tc.tile_pool(name="psum", bufs=2, space="PSUM") as pspool,
        ):
            wsb = cpool.tile([128, 128 * len(_VARIANTS)], F32, tag="w")
            bsb = cpool.tile([128, 16], F32, tag="b")
            nc.sync.dma_start(out=wsb[:], in_=wd[:])
            nc.sync.dma_start(out=bsb[:], in_=bd[:])

            def w_ap(name):
                _, _, K, M, _ = _VARIANTS[_VIDX[name]]
                c0 = 128 * _VIDX[name]
                return wsb[0:K, c0 : c0 + M]

            def b_ap(l, P):
                return bsb[0:P, l - 1 : l]

            def mm(ps_slice, wname, rhs, start=True, stop=True):
                _, _, K, M, _ = _VARIANTS[_VIDX[wname]]
                nc.tensor.matmul(ps_slice, w_ap(wname), rhs, start=start, stop=stop)

            def relu_scalar(h_slice, ps_slice, l, P):
                nc.scalar.activation(
                    h_slice, ps_slice, mybir.ActivationFunctionType.Relu,
                    bias=b_ap(l, P), scale=1.0,
                )

            def relu_vector(h_slice, ps_slice, l, P):
                nc.vector.tensor_scalar(
                    out=h_slice, in0=ps_slice,
                    scalar1=b_ap(l, P), scalar2=0.0,
                    op0=mybir.AluOpType.add, op1=mybir.AluOpType.max,
                )

            for st in range(NST):
                c0 = (S // 4) * st
                h0 = iopool.tile([60, 1024], F32, tag="h0")
                nc.sync.dma_start(out=h0[:], in_=xt[:, c0 : c0 + 1024])

                # L1: 15->30, 4-chunk packed
                ps = pspool.tile([128, 1024], F32, tag="ps")
                for t in range(2):
                    mm(ps[0:120, 512 * t : 512 * (t + 1)], "w1",
                       h0[0:60, 512 * t : 512 * (t + 1)])
                h1 = apool.tile([120, 1024], F32, tag="h1")
                relu_scalar(h1[:, :], ps[0:120, 0:1024], 1, 120)

                # L2: 30->60, A consumes chunks {0,1}, B chunks {2,3}
                ps = pspool.tile([128, 2048], F32, tag="ps")
                for t in range(2):
                    mm(ps[0:120, 512 * t : 512 * (t + 1)], "w2A",
                       h1[0:60, 512 * t : 512 * (t + 1)])
                    mm(ps[0:120, 1024 + 512 * t : 1024 + 512 * (t + 1)], "w2B",
                       h1[0:120, 512 * t : 512 * (t + 1)])
                h2 = apool.tile([120, 2048], F32, tag="h2")
                relu_vector(h2[:, :], ps[0:120, 0:2048], 2, 120)

                # L3: 60->90, A consumes chunk 0, B chunk 1
                h3 = apool.tile([90, 4096], F32, tag="h3")
                psA = pspool.tile([128, 2048], F32, tag="ps")
                for u in range(4):
                    mm(psA[0:90, 512 * u : 512 * (u + 1)], "w3A",
                       h2[0:60, 512 * u : 512 * (u + 1)])
                relu_scalar(h3[:, 0:2048], psA[0:90, 0:2048], 3, 90)
                psB = pspool.tile([128, 2048], F32, tag="ps")
                for u in range(4):
                    mm(psB[0:90, 512 * u : 512 * (u + 1)], "w3B",
                       h2[0:120, 512 * u : 512 * (u + 1)])
                relu_scalar(h3[:, 2048:4096], psB[0:90, 0:2048], 3, 90)

                # L4: 90->120
                h4 = apool.tile([120, 4096], F32, tag="h4")
                for r in range(2):
                    ps = pspool.tile([128, 2048], F32, tag="ps")
                    for q in range(4):
                        v = 4 * r + q
                        mm(ps[0:120, 512 * q : 512 * (q + 1)], "w4",
                           h3[0:90, 512 * v : 512 * (v + 1)])
                    relu_scalar(h4[:, 2048 * r : 2048 * (r + 1)], ps[0:120, 0:2048], 4, 120)

                # L5: 120->90
                h5 = apool.tile([90, 4096], F32, tag="h5")
                for r in range(2):
                    ps = pspool.tile([128, 2048], F32, tag="ps")
                    for q in range(4):
                        v = 4 * r + q
                        mm(ps[0:90, 512 * q : 512 * (q + 1)], "w5",
                           h4[0:120, 512 * v : 512 * (v + 1)])
                    relu_vector(h5[:, 2048 * r : 2048 * (r + 1)], ps[0:90, 0:2048], 5, 90)

                # L6: 90->60, pairs of tiles stack into one psum slice
                ps = pspool.tile([128, 2048], F32, tag="ps")
                for w in range(4):
                    sl = ps[0:60, 512 * w : 512 * (w + 1)]
                    mm(sl, "w6A", h5[0:90, 1024 * w : 1024 * w + 512],
                       start=True, stop=False)
                    sl2 = ps[0:120, 512 * w : 512 * (w + 1)]
                    mm(sl2, "w6B", h5[0:90, 1024 * w + 512 : 1024 * (w + 1)],
                       start=False, stop=True)
                h6 = apool.tile([120, 2048], F32, tag="h6")
                relu_vector(h6[:, :], ps[0:120, 0:2048], 6, 120)

                # L7: 60->30
                ps = pspool.tile([128, 1024], F32, tag="ps")
                for w in range(2):
                    mm(ps[0:60, 512 * w : 512 * (w + 1)], "w7A",
                       h6[0:120, 1024 * w : 1024 * w + 512], start=True, stop=False)
                    mm(ps[0:120, 512 * w : 512 * (w + 1)], "w7B",
                       h6[0:120, 1024 * w + 512 : 1024 * (w + 1)], start=False, stop=True)
                h7 = apool.tile([120, 1024], F32, tag="h7")
                relu_vector(h7[:, :], ps[0:120, 0:1024], 7, 120)

                # L8: 30->15
                ps = pspool.tile([128, 512], F32, tag="ps")
                mm(ps[0:60, :], "w8A", h7[0:120, 0:512], start=True, stop=False)
                mm(ps[0:120, :], "w8B", h7[0:120, 512:1024], start=False, stop=True)
                h8 = apool.tile([120, 512], F32, tag="h8")
                relu_scalar(h8[:, :], ps[0:120, 0:512], 8, 120)

                # L9: 15->10 (8-chunk packed)
                ps = pspool.tile([128, 512], F32, tag="ps")
                mm(ps[0:80, :], "w9", h8[0:120, :])
                h9 = apool.tile([80, 512], F32, tag="h9")
                relu_scalar(h9[:, :], ps[0:80, 0:512], 9, 80)

                # L10: 10->5
                ps = pspool.tile([128, 512], F32, tag="ps")
                mm(ps[0:40, :], "w10", h9[0:80, :])
                h10 = apool.tile([40, 512], F32, tag="h10")
                relu_scalar(h10[:, :], ps[0:40, 0:512], 10, 40)

                # L11: 5->1 + sigmoid
                ps = pspool.tile([128, 512], F32, tag="ps")
                mm(ps[0:8, :], "w11", h10[0:40, :])
                osb = iopool.tile([8, 512], F32, tag="osb")
                nc.scalar.activation(
                    osb[:, :], ps[0:8, 0:512], mybir.ActivationFunctionType.Sigmoid,
                    bias=b_ap(11, 8), scale=1.0,
                )
                nc.sync.dma_start(out=y[:, 512 * st : 512 * (st + 1)], in_=osb[:, :])

    nc.compile()
    _NC_CACHE = nc
    return nc


def kernel(**inputs):
    x = np.asarray(inputs["x"], dtype=np.float32)
    Ws = [np.asarray(inputs[f"W{i}"], dtype=np.float32) for i in range(1, 12)]
    bs = [np.asarray(inputs[f"b{i}"], dtype=np.float32) for i in range(1, 12)]

    w_pack = _pack_weights(Ws)
    b_pack = _pack_biases(bs)

    in_maps = []
    for c in range(NCORES):
        xc = x[c * BC : (c + 1) * BC]
        # xt[15j+f, (S//4)*st + m] = xc[st*S + j*(S//4) + m, f]
        xt = np.ascontiguousarray(
            xc.reshape(NST, 4, S // 4, _DIMS[0]).transpose(1, 3, 0, 2).reshape(60, BC // 4)
        )
        in_maps.append({"xt": xt, "w": w_pack, "b": b_pack})

    nc = _build_nc()
    res = run_bass_kernel_spmd(nc, in_maps, list(range(NCORES)))

    omap = _out_map()  # [8, 512] batch row within super-tile
    out = np.empty((B, 1), dtype=np.float32)
    for c in range(NCORES):
        yc = res.results[c]["y"]  # [8, BC//8]
        for st in range(NST):
            blk = np.empty(S, dtype=np.float32)
            blk[omap.ravel()] = yc[:, 512 * st : 512 * (st + 1)].ravel()
            out[c * BC + st * S : c * BC + (st + 1) * S, 0] = blk
    return out


# revision 18
# speedup vs baseline: 2804.4631x; 2804.4631x over previous
"""Trainium2 Bass kernel for DeepFunnelTransactionMLP.

MLP funnel 15->30->60->90->120->90->60->30->15->10->5->1 (ReLU between,
sigmoid at the end) over a batch of 524288 rows, fp32.

Strategy
--------
Pure data parallel: 8 cores x 65536 rows. On each core, activations are
kept feature-major (features on SBUF partitions, batch streaming on the
free dim), so every layer is one (or two) matmul(s) with the weight as
the stationary operand. Small layers are packed block-diagonally: e.g.
layer1 (15->30) processes 4 independent batch chunks in a single matmul
(4x15 input rows -> 4x30 output rows). Bias+ReLU are fused into single
ScalarE activation / VectorE tensor_scalar instructions reading PSUM.

Host side does the free work: transposing/packing x, building the
block-diagonal weights, and unpermuting the output.
"""

import sys

sys.path.insert(0, "/opt/trn_rl_repo")

import numpy as np

import concourse.bacc as bacc
import concourse.mybir as mybir
from concourse.bass_utils import run_bass_kernel_spmd
from concourse.tile import TileContext

_DIMS = [15, 30, 60, 90, 120, 90, 60, 30, 15, 10, 5, 1]
NCORES = 8
B = 524288
BC = B // NCORES  # 65536 rows per core
S = 4096  # super-tile rows
NST = BC // S  # 16 super-tiles per core
F32 = mybir.dt.float32
F32R = mybir.dt.float32r

# Weight variants: (layer l (1-based), K, M, [(koff, moff), ...]).
# lhsT[koff+k, moff+m] = W_l[m, k] for each block; matmul out = lhsT.T @ rhs.
_VARIANTS = [
    ("w1", 1, 60, 120, [(15 * j, 30 * j) for j in range(4)]),
    ("w2A", 2, 60, 120, [(0, 0), (30, 60)]),
    ("w2B", 2, 120, 120, [(60, 0), (90, 60)]),
    ("w3A", 3, 60, 90, [(0, 0)]),
    ("w3B", 3, 120, 90, [(60, 0)]),
    ("w4", 4, 90, 120, [(0, 0)]),
    ("w5", 5, 120, 90, [(0, 0)]),
    ("w6A", 6, 90, 60, [(0, 0)]),
    ("w6B", 6, 90, 120, [(0, 60)]),
    ("w7A", 7, 120, 60, [(0, 0), (60, 30)]),
    ("w7B", 7, 120, 120, [(0, 60), (60, 90)]),
    ("w8A", 8, 120, 60, [(30 * j, 15 * j) for j in range(4)]),
    ("w8B", 8, 120, 120, [(30 * j, 60 + 15 * j) for j in range(4)]),
    ("w9", 9, 120, 80, [(15 * j, 10 * j) for j in range(8)]),
    ("w10", 10, 80, 40, [(10 * j, 5 * j) for j in range(8)]),
    ("w11", 11, 40, 8, [(5 * j, j) for j in range(8)]),
]
_VIDX = {name: i for i, (name, *_) in enumerate(_VARIANTS)}

# Bias layouts: (layer l, tile count) -> packed [tile*dim] at column l-1.
_BIAS_TILES = [4, 2, 1, 1, 1, 2, 4, 8, 8, 8, 8]


def _pack_weights(Ws):
    w = np.zeros((128, 128 * len(_VARIANTS)), dtype=np.float32)
    for i, (_, l, K, M, blocks) in enumerate(_VARIANTS):
        Wl = Ws[l - 1]  # [fan_out, fan_in]
        fo, fi = Wl.shape
        for koff, moff in blocks:
            w[koff : koff + fi, 128 * i + moff : 128 * i + moff + fo] = Wl.T
    return w


def _pack_biases(bs):
    b = np.zeros((128, 16), dtype=np.float32)
    for l, (bl, nt) in enumerate(zip(bs, _BIAS_TILES)):
        v = np.tile(bl, nt)
        b[: v.shape[0], l] = v
    return b


def _out_map():
    """batch-row (within a super-tile) for output element [group j, col n]."""
    M0 = np.arange(S).reshape(4, S // 4)
    M1 = M0
    M2 = np.empty((2, 2048), dtype=np.int64)
    for t in range(2):
        M2[:, 512 * t : 512 * (t + 1)] = M1[0:2, 512 * t : 512 * (t + 1)]
        M2[:, 1024 + 512 * t : 1024 + 512 * (t + 1)] = M1[2:4, 512 * t : 512 * (t + 1)]
    M3 = np.empty((1, 4096), dtype=np.int64)
    for u in range(4):
        M3[0, 512 * u : 512 * (u + 1)] = M2[0, 512 * u : 512 * (u + 1)]
        M3[0, 2048 + 512 * u : 2048 + 512 * (u + 1)] = M2[1, 512 * u : 512 * (u + 1)]
    M5 = M3
    M6 = np.empty((2, 2048), dtype=np.int64)
    for w in range(4):
        M6[0, 512 * w : 512 * (w + 1)] = M5[0, 1024 * w : 1024 * w + 512]
        M6[1, 512 * w : 512 * (w + 1)] = M5[0, 1024 * w + 512 : 1024 * w + 1024]
    M7 = np.empty((4, 1024), dtype=np.int64)
    for w in range(2):
        M7[0:2, 512 * w : 512 * (w + 1)] = M6[0:2, 1024 * w : 1024 * w + 512]
        M7[2:4, 512 * w : 512 * (w + 1)] = M6[0:2, 1024 * w + 512 : 1024 * w + 1024]
    M8 = np.empty((8, 512), dtype=np.int64)
    M8[0:4, :] = M7[0:4, 0:512]
    M8[4:8, :] = M7[0:4, 512:1024]
    return M8


_NC_CACHE = None


def _build_nc():
    global _NC_CACHE
    if _NC_CACHE is not None:
        return _NC_CACHE

    nc = bacc.Bacc("TRN2", target_bir_lowering=False, debug=False, num_devices=NCORES)
    xt = nc.dram_tensor("xt", [60, BC // 4], F32R, kind="ExternalInput")
    wd = nc.dram_tensor("w", [128, 128 * len(_VARIANTS)], F32R, kind="ExternalInput")
    bd = nc.dram_tensor("b", [128, 16], F32, kind="ExternalInput")
    y = nc.dram_tensor("y", [8, BC // 8], F32, kind="ExternalOutput")

    with TileContext(nc) as tc:
        with (
            tc.tile_pool(name="const", bufs=1) as cpool,
            tc.tile_pool(name="act", bufs=1) as apool,
            tc.tile_pool(name="act2", bufs=2) as apool2,
            tc.tile_pool(name="io", bufs=3) as iopool,
            tc.tile_pool(name="psum", bufs=4, space="PSUM") as pspool,
        ):
            wsb = cpool.tile([128, 128 * len(_VARIANTS)], F32R, tag="w")
            bsb = cpool.tile([128, 16], F32, tag="b")
            nc.sync.dma_start(out=wsb[:], in_=wd[:])
            nc.sync.dma_start(out=bsb[:], in_=bd[:])

            def w_ap(name):
                _, _, K, M, _ = _VARIANTS[_VIDX[name]]
                c0 = 128 * _VIDX[name]
                return wsb[0:K, c0 : c0 + M]

            def b_ap(l, P):
                return bsb[0:P, l - 1 : l]

            eng_busy = [5500.0, 0.0]  # modeled ns on [ScalarE, VectorE]; ScalarE pre-charged to offset forced sigmoid work

            def round_(mms, h_slice, l, P, cols, force_scalar=False):
                """One psum tile: 512-col matmuls (slice advances on start=True),
                then a fused bias+relu drain on the less-loaded act engine."""
                ps = pspool.tile([128, 1024], F32, tag="ps")
                q = -1
                for wname, rhs, start, stop in mms:
                    _, _, K, M, _ = _VARIANTS[_VIDX[wname]]
                    if start:
                        q += 1
                    nc.tensor.matmul(ps[0:M, 512 * q : 512 * q + 512], w_ap(wname),
                                     rhs, start=start, stop=stop)
                if force_scalar or eng_busy[0] + cols / 1.2 + 143 <= eng_busy[1] + cols / 0.96 + 125:
                    nc.scalar.activation(
                        h_slice, ps[0:P, 0:cols], mybir.ActivationFunctionType.Relu,
                        bias=b_ap(l, P), scale=1.0,
                    )
                    eng_busy[0] += cols / 1.2 + 143
                else:
                    nc.vector.tensor_scalar(
                        out=h_slice, in0=ps[0:P, 0:cols],
                        scalar1=b_ap(l, P), scalar2=0.0,
                        op0=mybir.AluOpType.add, op1=mybir.AluOpType.max,
                    )
                    eng_busy[1] += cols / 0.96 + 125

            def build_t1(st, d):
                """L1-L3: 7 rounds."""
                rounds = []

                def r_dma():
                    c0 = (S // 4) * st
                    d["h0"] = iopool.tile([60, 1024], F32R, tag="h0", name="h0")
                    nc.sync.dma_start(out=d["h0"][:], in_=xt[:, c0 : c0 + 1024])
                    d["h1"] = apool2.tile([120, 1024], F32R, tag="h1", name="h1")
                    round_([("w1", d["h0"][0:60, 0:512], True, True),
                            ("w1", d["h0"][0:60, 512:1024], True, True)],
                           d["h1"][:, :], 1, 120, 1024)
                rounds.append(r_dma)

                def r_l2(half):
                    def f():
                        if half == 0:
                            d["h2"] = apool2.tile([120, 2048], F32R, tag="h2", name="h2")
                            round_([("w2A", d["h1"][0:60, 0:512], True, True),
                                    ("w2A", d["h1"][0:60, 512:1024], True, True)],
                                   d["h2"][:, 0:1024], 2, 120, 1024)
                        else:
                            round_([("w2B", d["h1"][0:120, 0:512], True, True),
                                    ("w2B", d["h1"][0:120, 512:1024], True, True)],
                                   d["h2"][:, 1024:2048], 2, 120, 1024)
                    return f
                rounds += [r_l2(0), r_l2(1)]

                def r_l3(r):
                    def f():
                        if r == 0:
                            d["h3"] = apool2.tile([90, 4096], F32R, tag="h3", name="h3")
                        if r < 2:
                            round_([("w3A", d["h2"][0:60, 1024 * r : 1024 * r + 512], True, True),
                                    ("w3A", d["h2"][0:60, 1024 * r + 512 : 1024 * (r + 1)], True, True)],
                                   d["h3"][:, 1024 * r : 1024 * (r + 1)], 3, 90, 1024)
                        else:
                            rr = r - 2
                            round_([("w3B", d["h2"][0:120, 1024 * rr : 1024 * rr + 512], True, True),
                                    ("w3B", d["h2"][0:120, 1024 * rr + 512 : 1024 * (rr + 1)], True, True)],
                                   d["h3"][:, 2048 + 1024 * rr : 2048 + 1024 * (rr + 1)], 3, 90, 1024)
                    return f
                rounds += [r_l3(r) for r in range(4)]
                return rounds

            def build_t2(st, d):
                """L4-L5: 8 rounds."""
                rounds = []

                def r_l4(r):
                    def f():
                        if r == 0:
                            d["h4"] = apool.tile([120, 4096], F32R, tag="h4", name="h4")
                        round_([("w4", d["h3"][0:90, 1024 * r : 1024 * r + 512], True, True),
                                ("w4", d["h3"][0:90, 1024 * r + 512 : 1024 * (r + 1)], True, True)],
                               d["h4"][:, 1024 * r : 1024 * (r + 1)], 4, 120, 1024)
                    return f
                rounds += [r_l4(r) for r in range(4)]

                def r_l5(r):
                    def f():
                        if r == 0:
                            d["h5"] = apool2.tile([90, 4096], F32R, tag="h5", name="h5")
                        round_([("w5", d["h4"][0:120, 1024 * r : 1024 * r + 512], True, True),
                                ("w5", d["h4"][0:120, 1024 * r + 512 : 1024 * (r + 1)], True, True)],
                               d["h5"][:, 1024 * r : 1024 * (r + 1)], 5, 90, 1024)
                    return f
                rounds += [r_l5(r) for r in range(4)]
                return rounds

            def build_t3(st, d):
                """L6-L11: 7 rounds."""
                rounds = []

                def r_l6(r):
                    def f():
                        if r == 0:
                            d["h6"] = apool2.tile([120, 2048], F32R, tag="h6", name="h6")
                        mms = []
                        for q in range(2):
                            w = 2 * r + q
                            mms.append(("w6A", d["h5"][0:90, 1024 * w : 1024 * w + 512], True, False))
                            mms.append(("w6B", d["h5"][0:90, 1024 * w + 512 : 1024 * (w + 1)], False, True))
                        round_(mms, d["h6"][:, 1024 * r : 1024 * (r + 1)], 6, 120, 1024)
                    return f
                rounds += [r_l6(r) for r in range(2)]

                def r_l7():
                    d["h7"] = apool2.tile([120, 1024], F32R, tag="h7", name="h7")
                    mms = []
                    for w in range(2):
                        mms.append(("w7A", d["h6"][0:120, 1024 * w : 1024 * w + 512], True, False))
                        mms.append(("w7B", d["h6"][0:120, 1024 * w + 512 : 1024 * (w + 1)], False, True))
                    round_(mms, d["h7"][:, :], 7, 120, 1024)
                rounds.append(r_l7)

                def r_l8():
                    d["h8"] = apool2.tile([120, 512], F32R, tag="h8", name="h8")
                    round_([("w8A", d["h7"][0:120, 0:512], True, False),
                            ("w8B", d["h7"][0:120, 512:1024], False, True)],
                           d["h8"][:, :], 8, 120, 512)
                rounds.append(r_l8)

                def r_l9():
                    d["h9"] = apool2.tile([80, 512], F32R, tag="h9", name="h9")
                    round_([("w9", d["h8"][0:120, :], True, True)], d["h9"][:, :], 9, 80, 512)
                rounds.append(r_l9)

                def r_l10():
                    d["h10"] = apool2.tile([40, 512], F32R, tag="h10", name="h10")
                    round_([("w10", d["h9"][0:80, :], True, True)], d["h10"][:, :], 10, 40, 512)
                rounds.append(r_l10)

                def r_l11():
                    ps = pspool.tile([128, 1024], F32, tag="ps", name="ps")
                    nc.tensor.matmul(ps[0:8, 0:512], w_ap("w11"), d["h10"][0:40, :],
                                     start=True, stop=True)
                    osb = iopool.tile([8, 512], F32, tag="osb", name="osb")
                    nc.scalar.activation(
                        osb[:, :], ps[0:8, 0:512], mybir.ActivationFunctionType.Sigmoid,
                        bias=b_ap(11, 8), scale=1.0,
                    )
                    eng_busy[0] += 512 / 1.2 + 143
                    nc.sync.dma_start(out=y[:, 512 * st : 512 * (st + 1)], in_=osb[:, :])
                rounds.append(r_l11)
                return rounds

            # 3-way software pipeline: epoch e runs T1(e), T2(e-1), T3(e-2)
            # round-robin, so three super-tiles' serial layer chains overlap
            # in every engine's in-order queue.
            from itertools import zip_longest

            dicts = [dict() for _ in range(NST)]
            for e in range(NST + 2):
                parts = []
                if e < NST:
                    parts.append(build_t1(e, dicts[e]))
                if 0 <= e - 1 < NST:
                    parts.append(build_t2(e - 1, dicts[e - 1]))
                if 0 <= e - 2 < NST:
                    parts.append(build_t3(e - 2, dicts[e - 2]))
                for grp in zip_longest(*parts):
                    for r in grp:
                        if r is not None:
                            r()

    nc.compile()
    _NC_CACHE = nc
    return nc


def _make_in_maps(inputs):
    x = np.asarray(inputs["x"], dtype=np.float32)
    Ws = [np.asarray(inputs[f"W{i}"], dtype=np.float32) for i in range(1, 12)]
    bs = [np.asarray(inputs[f"b{i}"], dtype=np.float32) for i in range(1, 12)]

    w_pack = _pack_weights(Ws)
    b_pack = _pack_biases(bs)

    in_maps = []
    for c in range(NCORES):
        xc = x[c * BC : (c + 1) * BC]
        # xt[15j+f, (S//4)*st + m] = xc[st*S + j*(S//4) + m, f]
        xt = np.ascontiguousarray(
            xc.reshape(NST, 4, S // 4, _DIMS[0]).transpose(1, 3, 0, 2).reshape(60, BC // 4)
        )
        in_maps.append({"xt": xt, "w": w_pack, "b": b_pack})
    return in_maps


def kernel(**inputs):
    in_maps = _make_in_maps(inputs)
    nc = _build_nc()
    res = run_bass_kernel_spmd(nc, in_maps, list(range(NCORES)))

    omap = _out_map()  # [8, 512] batch row within super-tile
    out = np.empty((B, 1), dtype=np.float32)
    for c in range(NCORES):
        yc = res.results[c]["y"]  # [8, BC//8]
        for st in range(NST):
            blk = np.empty(S, dtype=np.float32)
            blk[omap.ravel()] = yc[:, 512 * st : 512 * (st + 1)].ravel()
            out[c * BC + st * S : c * BC + (st + 1) * S, 0] = blk
    return out
